# revision 1
# baseline (speedup 1.0000x reference)
"""Trainium2 Bass kernel: CenterHeadIoU 1x1-conv stack.

Computes, for x = ct_feat [B=32, C=128, N=8192]:
  y = relu(bn(sh_w @ x))                       [B, 64, N]
  z_h = relu(bn_h(head_w1[h] @ y)), h=0..5     [B, 64, N] each
  out = concat_h(head_final_w[h] @ z_h + b_h)  [B, 12, N]

Sharding: data-parallel over batch, 4 batches per core on 8 cores;
weights are tiny and replicated. BN is folded into conv weights/biases
on the host. On device, per 512-column tile:
  mm1: lhsT [128,64] -> psum y [64, F]
  act1: relu(y + b1) PSUM->SBUF into y65[0:64] (row 64 holds constant 1.0)
  mm2 (x3): K=65 full-density pair weights with a bias row -> one
       [128,1536] psum tensor (chunk p = heads 2p/2p+1 stacked)
  z relu: one ACT op on [128,1024] + one DVE op on [128,512] (bias-free)
  mm3 (x3): accumulating matmuls (M=12, zero-padded pair blocks) into a
       dense [12, F] psum bank
  epi: DVE bias-add into a dense [12, N] per-batch accumulator; one DMA
       out per batch.
All matmuls run as float32r (full-rate fp32 mode, free dim 512).
A post-pass moves multi-wait sync conditions onto single-wait NoOp
carriers (this walrus build caps sync waits per instruction).
"""

import os
import sys
import numpy as np

B, C_IN, N, HC = 32, 128, 8192, 64
NCORES = 8
BC = B // NCORES            # batches per core
F = 512                     # free-dim tile = one fp32 PSUM bank
NT = N // F                 # tiles per batch
EPS = 1e-5
HEAD_OUT = [3, 2, 1, 3, 2, 1]        # hm, reg, height, dim, rot, iou
PAIR_OFF = [0, 5, 9]                 # channel offset of pair p in the 12-ch output

_CACHE = {}
LAST_RESULTS = None
LAST_EXEC_NS = None


def _build_program():
    import concourse.bass as bass
    import concourse.mybir as mybir
    import concourse.tile as tile

    f32 = mybir.dt.float32
    f32r = mybir.dt.float32r
    AF = mybir.ActivationFunctionType

    nc = bass.Bass("TRN2", target_bir_lowering=False, debug=False,
                   num_devices=NCORES)

    x = nc.dram_tensor("x", [BC, C_IN, N], f32r, kind="ExternalInput").ap()
    w1 = nc.dram_tensor("w1", [C_IN, HC], f32r, kind="ExternalInput").ap()
    b1 = nc.dram_tensor("b1", [HC, 1], f32, kind="ExternalInput").ap()
    w2 = nc.dram_tensor("w2", [HC + 1, 384], f32r, kind="ExternalInput").ap()
    w3 = nc.dram_tensor("w3", [128, 48], f32r, kind="ExternalInput").ap()
    b3 = nc.dram_tensor("b3", [12, 1], f32, kind="ExternalInput").ap()
    ones = nc.dram_tensor("ones", [1, F], f32r, kind="ExternalInput").ap()
    out = nc.dram_tensor("out", [BC, 12, N], f32, kind="ExternalOutput").ap()

    with tile.TileContext(nc) as tc:
        with (
            tc.tile_pool(name="consts", bufs=1) as cpool,
            tc.tile_pool(name="xin", bufs=4) as xpool,
            tc.tile_pool(name="ysb", bufs=1) as ypool,
            tc.tile_pool(name="zsb", bufs=2) as zpool,
            tc.tile_pool(name="osb", bufs=2) as opool,
            tc.tile_pool(name="py", bufs=1, space="PSUM") as pypool,
            tc.tile_pool(name="pz", bufs=2, space="PSUM") as pzpool,
            tc.tile_pool(name="po", bufs=1, space="PSUM") as popool,
        ):
            w1_t = cpool.tile([C_IN, HC], f32r, name="w1_t")
            nc.sync.dma_start(out=w1_t[:], in_=w1[:])
            b1_t = cpool.tile([HC, 1], f32, name="b1_t")
            nc.sync.dma_start(out=b1_t[:], in_=b1[:])
            w2_t = cpool.tile([HC + 1, 384], f32r, name="w2_t")
            nc.sync.dma_start(out=w2_t[:], in_=w2[:])
            w3_t = cpool.tile([128, 48], f32r, name="w3_t")
            nc.sync.dma_start(out=w3_t[:], in_=w3[:])
            b3_t = cpool.tile([12, 1], f32, name="b3_t")
            nc.sync.dma_start(out=b3_t[:], in_=b3[:])

            # y tiles hoisted: row 64 stays constant 1.0 (feeds the bias row
            # of the K=65 mm2), rows 0..63 rewritten by act1 each iteration.
            y_tiles = []
            for i in range(3):
                y65 = ypool.tile([HC + 1, F], f32r, name=f"y65_{i}",
                                 tag=f"y65_{i}")
                nc.sync.dma_start(out=y65[HC:HC + 1, :], in_=ones[:])
                y_tiles.append(y65)

            T = BC * NT

            def load_x(t):
                b, j = divmod(t, NT)
                xt = xpool.tile([C_IN, F], f32r, name="xt", tag="xt")
                nc.sync.dma_start(out=xt[:], in_=x[b, :, j * F:(j + 1) * F])
                return xt

            def mm1_act1(t, xt):
                py = pypool.tile([HC, F], f32, name="py", tag="py")
                nc.tensor.matmul(py[:], w1_t[:], xt[:], start=True, stop=True)
                y65 = y_tiles[t % 3]
                nc.scalar.activation(y65[0:HC, :], py[:], AF.Relu,
                                     bias=b1_t[:, 0:1], scale=1.0)
                return y65

            def mm3_epi(t, zt, ots):
                b, j = divmod(t, NT)
                po = popool.tile([12, F], f32, name="po", tag="po")
                for p in range(3):
                    nc.tensor.matmul(po[:, :],
                                     w3_t[:, 12 * p:12 * (p + 1)],
                                     zt[:, F * p:F * (p + 1)],
                                     start=(p == 0), stop=(p == 2))
                ot = ots[t]
                nc.vector.tensor_scalar_add(ot[:, j * F:(j + 1) * F],
                                            po[:, :], b3_t[:, 0:1])
                if j == NT - 1:
                    nc.sync.dma_start(out=out[b, :, :], in_=ot[:, :])

            # two-deep software pipeline: cycle t runs mm3/epi of tile t-1,
            # mm2/relu of tile t, and mm1/act1 of tile t+1, so the PE never
            # waits on the current tile's relu results.
            ots = {}
            xt = load_x(0)
            y65 = mm1_act1(0, xt)
            z_prev = None
            for t in range(T):
                b, j = divmod(t, NT)
                if j == 0:
                    ot = opool.tile([12, N], f32, name="ot", tag="ot")
                for jj in range(j, NT):
                    ots[b * NT + jj] = ot
                if t + 1 < T:
                    xt_next = load_x(t + 1)

                if z_prev is not None:
                    mm3_epi(t - 1, z_prev, ots)

                pz = pzpool.tile([128, 3 * F], f32, name="pz", tag="pz")
                for p in range(3):
                    nc.tensor.matmul(pz[:, F * p:F * (p + 1)],
                                     w2_t[:, 128 * p:128 * (p + 1)],
                                     y65[:, :],
                                     start=True, stop=True)
                zt = zpool.tile([128, 3 * F], f32r, name="zt", tag="zt")
                # bias already folded into mm2; pure relu
                nc.scalar.activation(zt[:, 0:2 * F], pz[:, 0:2 * F], AF.Relu)
                nc.vector.tensor_scalar_max(zt[:, 2 * F:3 * F],
                                            pz[:, 2 * F:3 * F], 0.0)

                if t + 1 < T:
                    y65 = mm1_act1(t + 1, xt_next)
                z_prev = zt
            mm3_epi(T - 1, z_prev, ots)
    _split_waits(nc)
    return nc


def _split_waits(nc, cap=1):
    """This container's walrus build rejects instructions carrying more than
    a small number of sync waits (fp32/f32r matmuls: just one). Move excess
    waits onto single-wait NoOp carriers inserted before the instruction on
    the same engine — semantically identical (conjunction of waits, in-order
    sequencers)."""
    import concourse.mybir as mybir

    k = 0
    for func in nc.m.functions:
        for bb in func.blocks:
            insts = bb.instructions
            out_insts = []
            changed = False
            for inst in insts:
                si = inst.sync_info
                waits = list(si.on_wait) if si and si.on_wait else []
                if len(waits) > cap:
                    for w in waits[:-cap]:
                        d = mybir.InstNoOp(name=f"I-sw{k}", ins=[], outs=[])
                        k += 1
                        d.engine = inst.engine
                        d.sync_info = mybir.SyncInfo(on_wait=[w], on_update=[])
                        nc.register_instruction(d)
                        out_insts.append(d)
                    inst.sync_info = mybir.SyncInfo(
                        on_wait=waits[-cap:],
                        on_update=list(si.on_update) if si.on_update else [])
                    changed = True
                out_insts.append(inst)
            if changed:
                bb.instructions = out_insts


def _get_program():
    if "nc" not in _CACHE:
        _CACHE["nc"] = _build_program()
    return _CACHE["nc"]


def _prep_weights(d):
    """Fold BN into conv weights/biases; pack stationary matrices."""
    f8 = np.float64

    def g(name):
        return np.asarray(d[name], dtype=f8)

    # shared conv + BN
    s1 = g("sh_g") / np.sqrt(g("sh_var") + EPS)                     # [64]
    W1e = g("sh_w") * s1[:, None]                                   # [64,128]
    b1e = g("sh_b") * s1 + g("sh_beta") - g("sh_mean") * s1         # [64]
    w1 = W1e.T.copy()                                               # [128,64]
    b1 = b1e[:, None]                                               # [64,1]

    # head first layers + BN: K=65 stationaries with a bias row; pair p's
    # block has head 2p in output cols 0..63 and head 2p+1 in cols 64..127.
    s2 = g("head_g1") / np.sqrt(g("head_var1") + EPS)               # [6,64]
    W2e = g("head_w1") * s2[:, :, None]                             # [6,64,64]
    b2e = g("head_b1") * s2 + g("head_beta1") - g("head_mean1") * s2  # [6,64]
    w2 = np.zeros((HC + 1, 384), f8)
    for p in range(3):
        w2[0:HC, 128 * p:128 * p + 64] = W2e[2 * p].T
        w2[0:HC, 128 * p + 64:128 * p + 128] = W2e[2 * p + 1].T
        w2[HC, 128 * p:128 * p + 64] = b2e[2 * p]
        w2[HC, 128 * p + 64:128 * p + 128] = b2e[2 * p + 1]

    # final convs: three accumulating M=12 blocks
    names = ["hm", "reg", "height", "dim", "rot", "iou"]
    Wf = [g(n + "_w") for n in names]
    bf = [g(n + "_b") for n in names]
    w3 = np.zeros((128, 48), f8)
    b3 = np.zeros((12, 1), f8)
    for p in range(3):
        ha, hb = 2 * p, 2 * p + 1
        ca, cb = HEAD_OUT[ha], HEAD_OUT[hb]
        off = PAIR_OFF[p]
        w3[0:64, 12 * p + off:12 * p + off + ca] = Wf[ha].T
        w3[64:128, 12 * p + off + ca:12 * p + off + ca + cb] = Wf[hb].T
        b3[off:off + ca, 0] = bf[ha]
        b3[off + ca:off + ca + cb, 0] = bf[hb]

    c = np.float32
    return {"w1": w1.astype(c), "b1": b1.astype(c), "w2": w2.astype(c),
            "w3": w3.astype(c), "b3": b3.astype(c),
            "ones": np.ones((1, F), np.float32)}


def _ensure_ntff_hook():
    """Install the antenv.axon_hooks NTFF-profile shim if the container's
    antenv package lacks it (profiling only; never used in grading runs)."""
    try:
        from antenv.axon_hooks import get_axon_ntff_profile_hook  # noqa: F401
        return True
    except ImportError:
        pass
    import contextlib
    import ctypes
    import sys as _sys
    import types

    so_path = "/opt/axon/libaxon_pjrt.so"
    if not os.path.exists(so_path):
        return False
    lib = ctypes.CDLL(so_path)
    if not hasattr(lib, "axon_start_nrt_profile"):
        return False
    lib.axon_start_nrt_profile.argtypes = [ctypes.POINTER(ctypes.c_int64),
                                           ctypes.c_size_t]
    lib.axon_start_nrt_profile.restype = ctypes.c_int64
    lib.axon_stop_nrt_profile.argtypes = [ctypes.c_char_p]
    lib.axon_stop_nrt_profile.restype = ctypes.c_int64

    @contextlib.contextmanager
    def _hook(output_dir, device_ids):
        import jax
        jax.devices()
        if device_ids:
            ids = (ctypes.c_int64 * len(device_ids))(*device_ids)
            rc = lib.axon_start_nrt_profile(ids, len(device_ids))
        else:
            rc = lib.axon_start_nrt_profile(None, 0)
        if rc != 0:
            raise RuntimeError(f"axon_start_nrt_profile rc={rc}")
        try:
            yield
        finally:
            n = lib.axon_stop_nrt_profile(str(output_dir).encode())
            print(f"profile: {n} file(s) written to {output_dir}",
                  file=sys.stderr)

    import antenv
    mod = types.ModuleType("antenv.axon_hooks")
    mod.get_axon_ntff_profile_hook = lambda: _hook
    mod.set_axon_ntff_profile_hook = lambda h: None
    _sys.modules["antenv.axon_hooks"] = mod
    antenv.axon_hooks = mod
    return True


def kernel(**inputs):
    global LAST_RESULTS, LAST_EXEC_NS
    from concourse.bass_utils import run_bass_kernel_spmd

    inputs = {k: np.asarray(v) for k, v in inputs.items()}
    weights = _prep_weights(inputs)

    ct = np.asarray(inputs["ct_feat"], dtype=np.float32)
    xs = ct.reshape(NCORES, BC, C_IN, N)

    in_maps = [dict(weights, x=np.ascontiguousarray(xs[i]))
               for i in range(NCORES)]

    nc = _get_program()
    trace = bool(int(os.environ.get("CK_PROFILE", "0")))
    if trace:
        trace = _ensure_ntff_hook()
    res = run_bass_kernel_spmd(nc, in_maps, list(range(NCORES)), trace=trace)
    LAST_RESULTS = res
    LAST_EXEC_NS = res.exec_time_ns

    out = np.concatenate([np.asarray(res.results[i]["out"])
                          for i in range(NCORES)], axis=0)
    return out.astype(np.float32)



# revision 2
# speedup vs baseline: 1.5011x; 1.5011x over previous
"""Trainium2 Bass kernel: CenterHeadIoU 1x1-conv stack (bf16 v2).

Computes, for x = ct_feat [B=32, C=128, N=8192]:
  y = relu(bn(sh_w @ x))                       [B, 64, N]
  z_h = relu(bn_h(head_w1[h] @ y)), h=0..5     [B, 64, N] each
  out = concat_h(head_final_w[h] @ z_h + b_h)  [B, 12, N]

Sharding: data-parallel over batch, 4 batches per core on 8 cores;
weights are tiny and replicated. BN folded into conv weights/biases on
the host; x and all weights cast to bf16 (verified rel err ~7e-4 vs the
2e-2 gate), which doubles the PE stream rate vs f32r and halves x DMA.

Per 512-column tile (F = one fp32 PSUM bank):
  mm1: lhsT [128,128] (cols 64..127 zero-padded so the tile stays in
       (128,128) mode) -> psum y [128, F]
  act1: relu(y + b1) PSUM->SBUF into y65[0:64] (row 64 holds 1.0)
  mm2 (x3): K=65 pair weights with a bias row -> psum z [128, 3F]
  z relu: ACT on bank 0, DVE max on banks 1-2 (different banks so the
       engines can touch PSUM concurrently)
Per group of 4 tiles:
  zero-fill matmul (K=128 zero weights, M=128) resets the out bank,
  then 12 accumulating matmuls (3 pairs x 4 tiles) in (128,32) mode at
  tile_position (0, 32*i) pack 4 tiles' [12, F] outputs into ONE psum
  bank -> a single bias-add evacuation op per 4 tiles (alternating
  ACT/DVE) -> 4 out DMAs.
PSUM: y 1 bank + z 2x3 banks + out 1 bank = exactly 8.
A post-pass moves multi-wait sync conditions onto single-wait NoOp
carriers (this walrus build caps sync waits per instruction).
"""

import os
import sys
import numpy as np

B, C_IN, N, HC = 32, 128, 8192, 64
NCORES = 8
BC = B // NCORES            # batches per core
F = 512                     # free-dim tile = one fp32 PSUM bank
NT = N // F                 # tiles per batch
T = BC * NT                 # tiles per core
G = 4                       # tiles per out-group (4 col positions)
NG = T // G
EPS = 1e-5
HEAD_OUT = [3, 2, 1, 3, 2, 1]        # hm, reg, height, dim, rot, iou
PAIR_OFF = [0, 5, 9]                 # channel offset of pair p in the 12-ch output

_CACHE = {}
LAST_RESULTS = None
LAST_EXEC_NS = None


def _build_program():
    import concourse.bass as bass
    import concourse.mybir as mybir
    import concourse.tile as tile

    f32 = mybir.dt.float32
    bf16 = mybir.dt.bfloat16
    AF = mybir.ActivationFunctionType

    nc = bass.Bass("TRN2", target_bir_lowering=False, debug=False,
                   num_devices=NCORES)

    x = nc.dram_tensor("x", [BC, C_IN, N], bf16, kind="ExternalInput").ap()
    w1 = nc.dram_tensor("w1", [C_IN, 128], bf16, kind="ExternalInput").ap()
    b1 = nc.dram_tensor("b1", [HC, 1], f32, kind="ExternalInput").ap()
    w2 = nc.dram_tensor("w2", [HC + 1, 384], bf16, kind="ExternalInput").ap()
    w3 = nc.dram_tensor("w3", [C_IN, 36], bf16, kind="ExternalInput").ap()
    wz = nc.dram_tensor("wz", [C_IN, 128], bf16, kind="ExternalInput").ap()
    b3 = nc.dram_tensor("b3", [128, 1], f32, kind="ExternalInput").ap()
    ones = nc.dram_tensor("ones", [1, F], bf16, kind="ExternalInput").ap()
    out = nc.dram_tensor("out", [BC, 12, N], f32, kind="ExternalOutput").ap()

    with tile.TileContext(nc) as tc:
        with (
            tc.tile_pool(name="consts", bufs=1) as cpool,
            tc.tile_pool(name="xin", bufs=3) as xpool,
            tc.tile_pool(name="ysb", bufs=1) as ypool,
            tc.tile_pool(name="zsb", bufs=6) as zpool,
            tc.tile_pool(name="osb", bufs=2) as opool,
            tc.tile_pool(name="py", bufs=1, space="PSUM") as pypool,
            tc.tile_pool(name="pz", bufs=2, space="PSUM") as pzpool,
            tc.tile_pool(name="po", bufs=1, space="PSUM") as popool,
        ):
            w1_t = cpool.tile([C_IN, 128], bf16, name="w1_t")
            nc.sync.dma_start(out=w1_t[:], in_=w1[:])
            b1_t = cpool.tile([HC, 1], f32, name="b1_t")
            nc.sync.dma_start(out=b1_t[:], in_=b1[:])
            w2_t = cpool.tile([HC + 1, 384], bf16, name="w2_t")
            nc.sync.dma_start(out=w2_t[:], in_=w2[:])
            w3_t = cpool.tile([C_IN, 36], bf16, name="w3_t")
            nc.sync.dma_start(out=w3_t[:], in_=w3[:])
            wz_t = cpool.tile([C_IN, 128], bf16, name="wz_t")
            nc.sync.dma_start(out=wz_t[:], in_=wz[:])
            b3_t = cpool.tile([128, 1], f32, name="b3_t")
            nc.sync.dma_start(out=b3_t[:], in_=b3[:])

            # y tiles hoisted: row 64 stays constant 1.0 (feeds the bias row
            # of the K=65 mm2), rows 0..63 rewritten by act1 each iteration.
            y_tiles = []
            for i in range(3):
                y65 = ypool.tile([HC + 1, F], bf16, name=f"y65_{i}",
                                 tag=f"y65_{i}")
                nc.sync.dma_start(out=y65[HC:HC + 1, :], in_=ones[:])
                y_tiles.append(y65)

            xts = {}

            def load_group(g):
                if g >= NG:
                    return
                b, jg = divmod(g, NT // G)
                xt = xpool.tile([C_IN, G * F], bf16, name="xt", tag="xt")
                nc.sync.dma_start(out=xt[:],
                                  in_=x[b, :, jg * G * F:(jg + 1) * G * F])
                xts[g] = xt

            def mm1_act1(t):
                g, i = divmod(t, G)
                py = pypool.tile([C_IN, F], f32, name="py", tag="py")
                nc.tensor.matmul(py[:], w1_t[:],
                                 xts[g][:, i * F:(i + 1) * F],
                                 start=True, stop=True)
                y65 = y_tiles[t % 3]
                nc.scalar.activation(y65[0:HC, :], py[0:HC, :], AF.Relu,
                                     bias=b1_t[:, 0:1], scale=1.0)

            def mm2(t):
                y65 = y_tiles[t % 3]
                pz = pzpool.tile([C_IN, 3 * F], f32, name="pz", tag="pz")
                for p in range(3):
                    nc.tensor.matmul(pz[:, F * p:F * (p + 1)],
                                     w2_t[:, 128 * p:128 * (p + 1)],
                                     y65[:, :], start=True, stop=True)
                return pz

            def act2(t, pz):
                zs = zpool.tile([C_IN, 3 * F], bf16, name="zs", tag="zs")
                # bias folded into mm2's K=65 row; pure relu, split on a
                # PSUM bank boundary between ACT and DVE
                nc.scalar.activation(zs[:, 0:F], pz[:, 0:F], AF.Relu)
                nc.vector.tensor_scalar_max(zs[:, F:3 * F], pz[:, F:3 * F],
                                            0.0)
                return zs

            def mm3_group(g, zss):
                po = popool.tile([C_IN, F], f32, name="po", tag="po")
                # zero-fill the whole bank (clears has_written everywhere)
                # so the 12 col-tiled matmuls below can all accumulate with
                # start=False regardless of bank- vs region-clear semantics.
                nc.tensor.matmul(po[:, :], wz_t[:], xts[g][:, 0:F],
                                 start=True, stop=True, skip_group_check=True)
                for p in range(3):
                    for i in range(G):
                        nc.tensor.matmul(
                            po[32 * i:32 * i + 12, :],
                            w3_t[:, 12 * p:12 * (p + 1)],
                            zss[i][:, F * p:F * (p + 1)],
                            start=False, stop=(p == 2 and i == G - 1),
                            skip_group_check=True,
                            tile_position=(0, 32 * i))
                return po

            def epi_group(g, po):
                b, jg = divmod(g, NT // G)
                ot = opool.tile([128, F], f32, name="ot", tag="ot")
                if g % 2 == 0:
                    nc.vector.tensor_scalar_add(ot[:, :], po[:, :],
                                                b3_t[:, 0:1])
                else:
                    nc.scalar.activation(ot[:, :], po[:, :], AF.Identity,
                                         bias=b3_t[:, 0:1], scale=1.0)
                for i in range(G):
                    j = jg * G + i
                    nc.sync.dma_start(out=out[b, :, j * F:(j + 1) * F],
                                      in_=ot[32 * i:32 * i + 12, :])

            # two-tile software pipeline: cycle t runs mm2 of tile t (using
            # y from act1 two cycles back) then mm1+act1 of tile t+2, so the
            # PE never waits on the current tile's act1.
            load_group(0)
            load_group(1)
            mm1_act1(0)
            mm1_act1(1)
            zss = {}
            for t in range(T):
                g, i = divmod(t, G)
                if i == 0:
                    load_group(g + 2)
                pz = mm2(t)
                if t + 2 < T:
                    mm1_act1(t + 2)
                zss[t] = act2(t, pz)
                if i == G - 1:
                    po = mm3_group(g, [zss[G * g + k] for k in range(G)])
                    epi_group(g, po)
    _split_waits(nc)
    return nc


def _split_waits(nc, cap=1):
    """This container's walrus build rejects instructions carrying more than
    a small number of sync waits (matmuls: just one). Move excess waits onto
    single-wait NoOp carriers inserted before the instruction on the same
    engine — semantically identical (conjunction of waits, in-order
    sequencers)."""
    import concourse.mybir as mybir

    k = 0
    for func in nc.m.functions:
        for bb in func.blocks:
            insts = bb.instructions
            out_insts = []
            changed = False
            for inst in insts:
                si = inst.sync_info
                waits = list(si.on_wait) if si and si.on_wait else []
                if len(waits) > cap:
                    for w in waits[:-cap]:
                        d = mybir.InstNoOp(name=f"I-sw{k}", ins=[], outs=[])
                        k += 1
                        d.engine = inst.engine
                        d.sync_info = mybir.SyncInfo(on_wait=[w], on_update=[])
                        nc.register_instruction(d)
                        out_insts.append(d)
                    inst.sync_info = mybir.SyncInfo(
                        on_wait=waits[-cap:],
                        on_update=list(si.on_update) if si.on_update else [])
                    changed = True
                out_insts.append(inst)
            if changed:
                bb.instructions = out_insts


def _get_program():
    if "nc" not in _CACHE:
        _CACHE["nc"] = _build_program()
    return _CACHE["nc"]


def _prep_weights(d):
    """Fold BN into conv weights/biases; pack stationary matrices (bf16)."""
    import ml_dtypes
    f8 = np.float64
    bf = ml_dtypes.bfloat16

    def g(name):
        return np.asarray(d[name], dtype=f8)

    # shared conv + BN
    s1 = g("sh_g") / np.sqrt(g("sh_var") + EPS)                     # [64]
    W1e = g("sh_w") * s1[:, None]                                   # [64,128]
    b1e = g("sh_b") * s1 + g("sh_beta") - g("sh_mean") * s1         # [64]
    w1 = np.zeros((C_IN, 128), f8)
    w1[:, 0:HC] = W1e.T                 # cols 64..127 zero: M padded to 128
    b1 = b1e[:, None]                                               # [64,1]

    # head first layers + BN: K=65 stationaries with a bias row; pair p's
    # block has head 2p in output cols 0..63 and head 2p+1 in cols 64..127.
    s2 = g("head_g1") / np.sqrt(g("head_var1") + EPS)               # [6,64]
    W2e = g("head_w1") * s2[:, :, None]                             # [6,64,64]
    b2e = g("head_b1") * s2 + g("head_beta1") - g("head_mean1") * s2  # [6,64]
    w2 = np.zeros((HC + 1, 384), f8)
    for p in range(3):
        w2[0:HC, 128 * p:128 * p + 64] = W2e[2 * p].T
        w2[0:HC, 128 * p + 64:128 * p + 128] = W2e[2 * p + 1].T
        w2[HC, 128 * p:128 * p + 64] = b2e[2 * p]
        w2[HC, 128 * p + 64:128 * p + 128] = b2e[2 * p + 1]

    # final convs: three accumulating M=12 blocks (pair p: head 2p from z
    # rows 0..63, head 2p+1 from rows 64..127, into its channel offsets)
    names = ["hm", "reg", "height", "dim", "rot", "iou"]
    Wf = [g(n + "_w") for n in names]
    bfin = [g(n + "_b") for n in names]
    w3 = np.zeros((C_IN, 36), f8)
    b3full = np.zeros((12,), f8)
    for p in range(3):
        ha, hb = 2 * p, 2 * p + 1
        ca, cb = HEAD_OUT[ha], HEAD_OUT[hb]
        off = PAIR_OFF[p]
        w3[0:64, 12 * p + off:12 * p + off + ca] = Wf[ha].T
        w3[64:128, 12 * p + off + ca:12 * p + off + ca + cb] = Wf[hb].T
        b3full[off:off + ca] = bfin[ha]
        b3full[off + ca:off + ca + cb] = bfin[hb]
    b3 = np.zeros((128, 1), f8)
    for i in range(G):
        b3[32 * i:32 * i + 12, 0] = b3full

    c = np.float32
    return {"w1": w1.astype(bf), "b1": b1.astype(c), "w2": w2.astype(bf),
            "w3": w3.astype(bf), "wz": np.zeros((C_IN, 128), bf),
            "b3": b3.astype(c), "ones": np.ones((1, F), bf)}


def _ensure_ntff_hook():
    """Install the antenv.axon_hooks NTFF-profile shim if the container's
    antenv package lacks it (profiling only; never used in grading runs)."""
    try:
        from antenv.axon_hooks import get_axon_ntff_profile_hook  # noqa: F401
        return True
    except ImportError:
        pass
    import contextlib
    import ctypes
    import sys as _sys
    import types

    so_path = "/opt/axon/libaxon_pjrt.so"
    if not os.path.exists(so_path):
        return False
    lib = ctypes.CDLL(so_path)
    if not hasattr(lib, "axon_start_nrt_profile"):
        return False
    lib.axon_start_nrt_profile.argtypes = [ctypes.POINTER(ctypes.c_int64),
                                           ctypes.c_size_t]
    lib.axon_start_nrt_profile.restype = ctypes.c_int64
    lib.axon_stop_nrt_profile.argtypes = [ctypes.c_char_p]
    lib.axon_stop_nrt_profile.restype = ctypes.c_int64

    @contextlib.contextmanager
    def _hook(output_dir, device_ids):
        import jax
        jax.devices()
        if device_ids:
            ids = (ctypes.c_int64 * len(device_ids))(*device_ids)
            rc = lib.axon_start_nrt_profile(ids, len(device_ids))
        else:
            rc = lib.axon_start_nrt_profile(None, 0)
        if rc != 0:
            raise RuntimeError(f"axon_start_nrt_profile rc={rc}")
        try:
            yield
        finally:
            n = lib.axon_stop_nrt_profile(str(output_dir).encode())
            print(f"profile: {n} file(s) written to {output_dir}",
                  file=sys.stderr)

    import antenv
    mod = types.ModuleType("antenv.axon_hooks")
    mod.get_axon_ntff_profile_hook = lambda: _hook
    mod.set_axon_ntff_profile_hook = lambda h: None
    _sys.modules["antenv.axon_hooks"] = mod
    antenv.axon_hooks = mod
    return True


def kernel(**inputs):
    global LAST_RESULTS, LAST_EXEC_NS
    import ml_dtypes
    from concourse.bass_utils import run_bass_kernel_spmd

    inputs = {k: np.asarray(v) for k, v in inputs.items()}
    weights = _prep_weights(inputs)

    ct = np.asarray(inputs["ct_feat"], dtype=np.float32)
    xs = ct.astype(ml_dtypes.bfloat16).reshape(NCORES, BC, C_IN, N)

    in_maps = [dict(weights, x=np.ascontiguousarray(xs[i]))
               for i in range(NCORES)]

    nc = _get_program()
    trace = bool(int(os.environ.get("CK_PROFILE", "0")))
    if trace:
        trace = _ensure_ntff_hook()
    res = run_bass_kernel_spmd(nc, in_maps, list(range(NCORES)), trace=trace)
    LAST_RESULTS = res
    LAST_EXEC_NS = res.exec_time_ns

    out = np.concatenate([np.asarray(res.results[i]["out"])
                          for i in range(NCORES)], axis=0)
    return out.astype(np.float32)


# revision 3
# speedup vs baseline: 2.4031x; 1.6009x over previous
"""Trainium2 Bass kernel: CenterHeadIoU 1x1-conv stack (bf16 v3, PE-tiled).

Computes, for x = ct_feat [B=32, C=128, N=8192]:
  y = relu(bn(sh_w @ x))                       [B, 64, N]
  z_h = relu(bn_h(head_w1[h] @ y)), h=0..5     [B, 64, N] each
  out = concat_h(head_final_w[h] @ z_h + b_h)  [B, 12, N]

Sharding: data-parallel over batch, 4 batches per core on 8 cores;
weights tiny and replicated. BN folded on the host; x and weights cast
to bf16 (rel err ~7e-4 vs the 2e-2 gate). On this part the PE streams
512 columns in ~427ns regardless of dtype (1.2 GHz), so the design
minimizes full-width matmul slots via PE array tiling (concurrent
tiles at distinct tile_positions):

Per group of 4 tiles (F=512 cols each), three mode stretches:
  (128,64): mm1 for the NEXT group as 2 col-pairs — tiles t,t+1 share
       one slot writing psum partitions 0-63 / 64-127; one ACT op then
       evacuates BOTH tiles' y (relu+bias) into a [128,F] sbuf tile.
  (128,32): mm3 for the PREVIOUS group — 3 waves of 4 col-tiled
       accumulating matmuls (M=12 at positions 0/32/64/96) pack 4
       tiles' outputs into ONE psum bank (z is a group old, so no PE
       stall); a single copy op evacuates it, then 4 out-DMAs on the
       idle GpSimd queue.
  (64,128): mm2 — pair p of tile t runs on row half t%2 (y(t) lives in
       sbuf partitions 0-63 or 64-127), so two tiles' pair-matmuls run
       concurrently: 12 matmuls in ~6 slots. K=64 (no bias row): the
       first-layer bias is applied by the PSUM-evac ops (per-partition
       bias AP). The stretch ends with a rank-1 "bias matmul"
       (K=64 onesK row) that fills the NEXT out bank with b3 and sets
       has_written everywhere, so mm3 waves accumulate with
       start=False and the final evac is a plain copy.
All 15 PSUM allocations per group share one 8-bank ring (tag "ps").
A post-pass moves multi-wait sync conditions onto single-wait NoOp
carriers (this walrus build caps sync waits per instruction).
"""

import os
import sys
import numpy as np

B, C_IN, N, HC = 32, 128, 8192, 64
NCORES = 8
BC = B // NCORES            # batches per core
F = 512                     # free-dim tile = one fp32 PSUM bank
NT = N // F                 # tiles per batch
T = BC * NT                 # tiles per core
G = 4                       # tiles per out-group (4 col positions)
NG = T // G
EPS = 1e-5
HEAD_OUT = [3, 2, 1, 3, 2, 1]        # hm, reg, height, dim, rot, iou
PAIR_OFF = [0, 5, 9]                 # channel offset of pair p in the 12-ch output

_CACHE = {}
LAST_RESULTS = None
LAST_EXEC_NS = None


def _build_program():
    import concourse.bass as bass
    import concourse.mybir as mybir
    import concourse.tile as tile

    f32 = mybir.dt.float32
    bf16 = mybir.dt.bfloat16
    AF = mybir.ActivationFunctionType
    OP = mybir.AluOpType

    nc = bass.Bass("TRN2", target_bir_lowering=False, debug=False,
                   num_devices=NCORES)

    x = nc.dram_tensor("x", [BC, C_IN, N], bf16, kind="ExternalInput").ap()
    w1 = nc.dram_tensor("w1", [C_IN, 128], bf16, kind="ExternalInput").ap()
    b1 = nc.dram_tensor("b1", [128, 1], f32, kind="ExternalInput").ap()
    w2 = nc.dram_tensor("w2", [C_IN, 384], bf16, kind="ExternalInput").ap()
    b2 = nc.dram_tensor("b2", [128, 3], f32, kind="ExternalInput").ap()
    w3 = nc.dram_tensor("w3", [C_IN, 36], bf16, kind="ExternalInput").ap()
    b3r = nc.dram_tensor("b3r", [HC, 128], bf16, kind="ExternalInput").ap()
    onesk = nc.dram_tensor("onesk", [HC, F], bf16, kind="ExternalInput").ap()
    out = nc.dram_tensor("out", [BC, 12, N], f32, kind="ExternalOutput").ap()

    with tile.TileContext(nc) as tc:
        with (
            tc.tile_pool(name="consts", bufs=1) as cpool,
            tc.tile_pool(name="xin", bufs=3) as xpool,
            tc.tile_pool(name="ysb", bufs=2) as ypool,
            tc.tile_pool(name="zsb", bufs=9) as zpool,
            tc.tile_pool(name="osb", bufs=2) as opool,
            tc.tile_pool(name="ps", bufs=8, space="PSUM") as ppool,
        ):
            w1_t = cpool.tile([C_IN, 128], bf16, name="w1_t")
            nc.sync.dma_start(out=w1_t[:], in_=w1[:])
            b1_t = cpool.tile([128, 1], f32, name="b1_t")
            nc.sync.dma_start(out=b1_t[:], in_=b1[:])
            w2_t = cpool.tile([C_IN, 384], bf16, name="w2_t")
            nc.sync.dma_start(out=w2_t[:], in_=w2[:])
            b2_t = cpool.tile([128, 3], f32, name="b2_t")
            nc.sync.dma_start(out=b2_t[:], in_=b2[:])
            w3_t = cpool.tile([C_IN, 36], bf16, name="w3_t")
            nc.sync.dma_start(out=w3_t[:], in_=w3[:])
            b3r_t = cpool.tile([HC, 128], bf16, name="b3r_t")
            nc.sync.dma_start(out=b3r_t[:], in_=b3r[:])
            onesk_t = cpool.tile([HC, F], bf16, name="onesk_t")
            nc.sync.dma_start(out=onesk_t[:], in_=onesk[:])

            xts = {}
            yts = {}
            zss = {}
            pos = {}

            def psum():
                return ppool.tile([C_IN, F], f32, name="ps", tag="ps")

            def load_group(g):
                if g >= NG:
                    return
                b, jg = divmod(g, NT // G)
                xt = xpool.tile([C_IN, G * F], bf16, name="xt", tag="xt")
                nc.sync.dma_start(out=xt[:],
                                  in_=x[b, :, jg * G * F:(jg + 1) * G * F])
                xts[g] = xt

            def mm1_group(g):
                # (128,64) stretch: y for group g, two tiles per slot via
                # col tiling; one ACT evac per slot covers both tiles.
                if g >= NG:
                    return
                xt = xts[g]
                ys = []
                for pair in range(2):       # tiles (2*pair, 2*pair+1)
                    py = psum()
                    nc.tensor.matmul(py[0:HC, :], w1_t[:, 0:HC],
                                     xt[:, (2 * pair) * F:(2 * pair + 1) * F],
                                     start=True, stop=True)
                    nc.tensor.matmul(py[HC:128, :], w1_t[:, HC:128],
                                     xt[:, (2 * pair + 1) * F:(2 * pair + 2) * F],
                                     start=True, stop=True)
                    yt = ypool.tile([C_IN, F], bf16, name=f"y{pair}",
                                    tag=f"y{pair}")
                    nc.scalar.activation(yt[:, :], py[:, :], AF.Relu,
                                         bias=b1_t[:, 0:1], scale=1.0)
                    ys.append(yt)
                yts[g] = ys

            def mm2_group(g):
                # (64,128) stretch: pair-matmuls for tiles 4g..4g+3; tile
                # parity picks the row half, so consecutive tiles overlap.
                for i in range(G):
                    t = G * g + i
                    yt = yts[g][i // 2]
                    h = HC * (i % 2)
                    pzs = []
                    for p in range(3):
                        pz = psum()
                        nc.tensor.matmul(pz[:, :],
                                         w2_t[h:h + HC, 128 * p:128 * (p + 1)],
                                         yt[h:h + HC, :],
                                         start=True, stop=True)
                        pzs.append(pz)
                    zs = zpool.tile([C_IN, 3 * F], bf16, name="zs", tag="zs")
                    # relu + first-layer bias via per-partition bias APs;
                    # per-bank ops so ACT and DVE touch different banks
                    nc.scalar.activation(zs[:, 0:F], pzs[0][:, :], AF.Relu,
                                         bias=b2_t[:, 0:1], scale=1.0)
                    nc.vector.tensor_scalar(zs[:, F:2 * F], pzs[1][:, :],
                                            b2_t[:, 1:2], 0.0,
                                            OP.add, OP.max)
                    if i % 2 == 0:
                        nc.scalar.activation(zs[:, 2 * F:3 * F], pzs[2][:, :],
                                             AF.Relu, bias=b2_t[:, 2:3],
                                             scale=1.0)
                    else:
                        nc.vector.tensor_scalar(zs[:, 2 * F:3 * F],
                                                pzs[2][:, :], b2_t[:, 2:3],
                                                0.0, OP.add, OP.max)
                    zss[t] = zs
                # rank-1 bias matmul: fills the out bank for group g's mm3
                # with b3 and sets has_written on every element
                po = psum()
                nc.tensor.matmul(po[:, :], b3r_t[:, :], onesk_t[:, :],
                                 start=True, stop=True, skip_group_check=True)
                pos[g] = po

            def mm3_group(g):
                # (128,32) stretch: 3 waves of 4 col-tiled accumulating
                # matmuls; z is one group old so the PE never waits here.
                po = pos.pop(g)
                for p in range(3):
                    for i in range(G):
                        t = G * g + i
                        nc.tensor.matmul(
                            po[32 * i:32 * i + 12, :],
                            w3_t[:, 12 * p:12 * (p + 1)],
                            zss[t][:, F * p:F * (p + 1)],
                            start=False, stop=(p == 2 and i == G - 1),
                            skip_group_check=True,
                            tile_position=(0, 32 * i))
                return po

            def epi_group(g, po):
                b, jg = divmod(g, NT // G)
                ot = opool.tile([128, F], f32, name="ot", tag="ot")
                if g % 2 == 0:
                    nc.vector.tensor_scalar_add(ot[:, :], po[:, :], 0.0)
                else:
                    nc.scalar.activation(ot[:, :], po[:, :], AF.Copy)
                for i in range(G):
                    j = jg * G + i
                    nc.gpsimd.dma_start(out=out[b, :, j * F:(j + 1) * F],
                                        in_=ot[32 * i:32 * i + 12, :])
                for t in range(G * g, G * g + G):
                    zss.pop(t, None)

            load_group(0)
            load_group(1)
            mm1_group(0)
            for g in range(NG):
                load_group(g + 2)
                mm1_group(g + 1)
                if g >= 1:
                    po = mm3_group(g - 1)
                    epi_group(g - 1, po)
                mm2_group(g)
            po = mm3_group(NG - 1)
            epi_group(NG - 1, po)
    _split_waits(nc)
    return nc


def _split_waits(nc, cap=1):
    """This container's walrus build rejects instructions carrying more than
    a small number of sync waits (matmuls: just one). Move excess waits onto
    single-wait NoOp carriers inserted before the instruction on the same
    engine — semantically identical (conjunction of waits, in-order
    sequencers)."""
    import concourse.mybir as mybir

    k = 0
    for func in nc.m.functions:
        for bb in func.blocks:
            insts = bb.instructions
            out_insts = []
            changed = False
            for inst in insts:
                si = inst.sync_info
                waits = list(si.on_wait) if si and si.on_wait else []
                if len(waits) > cap:
                    for w in waits[:-cap]:
                        d = mybir.InstNoOp(name=f"I-sw{k}", ins=[], outs=[])
                        k += 1
                        d.engine = inst.engine
                        d.sync_info = mybir.SyncInfo(on_wait=[w], on_update=[])
                        nc.register_instruction(d)
                        out_insts.append(d)
                    inst.sync_info = mybir.SyncInfo(
                        on_wait=waits[-cap:],
                        on_update=list(si.on_update) if si.on_update else [])
                    changed = True
                out_insts.append(inst)
            if changed:
                bb.instructions = out_insts


def _get_program():
    if "nc" not in _CACHE:
        _CACHE["nc"] = _build_program()
    return _CACHE["nc"]


def _prep_weights(d):
    """Fold BN into conv weights/biases; pack stationary matrices (bf16)."""
    import ml_dtypes
    f8 = np.float64
    bf = ml_dtypes.bfloat16

    def g(name):
        return np.asarray(d[name], dtype=f8)

    # shared conv + BN
    s1 = g("sh_g") / np.sqrt(g("sh_var") + EPS)                     # [64]
    W1e = g("sh_w") * s1[:, None]                                   # [64,128]
    b1e = g("sh_b") * s1 + g("sh_beta") - g("sh_mean") * s1         # [64]
    w1 = np.zeros((C_IN, 128), f8)
    w1[:, 0:HC] = W1e.T                 # col-pair slot 0 -> psum rows 0..63
    w1[:, HC:128] = W1e.T               # col-pair slot 1 -> psum rows 64..127
    b1 = np.concatenate([b1e, b1e])[:, None]                        # [128,1]

    # head first layers + BN: K=64 row-tiled stationaries; pair p's block
    # has head 2p in output cols 0..63 and head 2p+1 in cols 64..127; the
    # weights are duplicated into both sbuf row halves (T0 and T8 copies).
    s2 = g("head_g1") / np.sqrt(g("head_var1") + EPS)               # [6,64]
    W2e = g("head_w1") * s2[:, :, None]                             # [6,64,64]
    b2e = g("head_b1") * s2 + g("head_beta1") - g("head_mean1") * s2  # [6,64]
    w2 = np.zeros((C_IN, 384), f8)
    b2 = np.zeros((128, 3), f8)
    for p in range(3):
        blk = np.zeros((HC, 128), f8)
        blk[:, 0:HC] = W2e[2 * p].T
        blk[:, HC:128] = W2e[2 * p + 1].T
        w2[0:HC, 128 * p:128 * (p + 1)] = blk
        w2[HC:128, 128 * p:128 * (p + 1)] = blk
        b2[0:HC, p] = b2e[2 * p]
        b2[HC:128, p] = b2e[2 * p + 1]

    # final convs: three accumulating M=12 blocks (pair p: head 2p from z
    # rows 0..63, head 2p+1 from rows 64..127, into its channel offsets)
    names = ["hm", "reg", "height", "dim", "rot", "iou"]
    Wf = [g(n + "_w") for n in names]
    bfin = [g(n + "_b") for n in names]
    w3 = np.zeros((C_IN, 36), f8)
    b3full = np.zeros((12,), f8)
    for p in range(3):
        ha, hb = 2 * p, 2 * p + 1
        ca, cb = HEAD_OUT[ha], HEAD_OUT[hb]
        off = PAIR_OFF[p]
        w3[0:64, 12 * p + off:12 * p + off + ca] = Wf[ha].T
        w3[64:128, 12 * p + off + ca:12 * p + off + ca + cb] = Wf[hb].T
        b3full[off:off + ca] = bfin[ha]
        b3full[off + ca:off + ca + cb] = bfin[hb]
    # rank-1 bias matmul operands: row 0 of b3r x row 0 of onesk broadcasts
    # b3 (in the 4x32 col-group layout) across the whole out bank
    b3r = np.zeros((HC, 128), f8)
    for i in range(G):
        b3r[0, 32 * i:32 * i + 12] = b3full
    onesk = np.zeros((HC, F), f8)
    onesk[0, :] = 1.0

    c = np.float32
    return {"w1": w1.astype(bf), "b1": b1.astype(c), "w2": w2.astype(bf),
            "b2": b2.astype(c), "w3": w3.astype(bf), "b3r": b3r.astype(bf),
            "onesk": onesk.astype(bf)}


def _ensure_ntff_hook():
    """Install the antenv.axon_hooks NTFF-profile shim if the container's
    antenv package lacks it (profiling only; never used in grading runs)."""
    try:
        from antenv.axon_hooks import get_axon_ntff_profile_hook  # noqa: F401
        return True
    except ImportError:
        pass
    import contextlib
    import ctypes
    import sys as _sys
    import types

    so_path = "/opt/axon/libaxon_pjrt.so"
    if not os.path.exists(so_path):
        return False
    lib = ctypes.CDLL(so_path)
    if not hasattr(lib, "axon_start_nrt_profile"):
        return False
    lib.axon_start_nrt_profile.argtypes = [ctypes.POINTER(ctypes.c_int64),
                                           ctypes.c_size_t]
    lib.axon_start_nrt_profile.restype = ctypes.c_int64
    lib.axon_stop_nrt_profile.argtypes = [ctypes.c_char_p]
    lib.axon_stop_nrt_profile.restype = ctypes.c_int64

    @contextlib.contextmanager
    def _hook(output_dir, device_ids):
        import jax
        jax.devices()
        if device_ids:
            ids = (ctypes.c_int64 * len(device_ids))(*device_ids)
            rc = lib.axon_start_nrt_profile(ids, len(device_ids))
        else:
            rc = lib.axon_start_nrt_profile(None, 0)
        if rc != 0:
            raise RuntimeError(f"axon_start_nrt_profile rc={rc}")
        try:
            yield
        finally:
            n = lib.axon_stop_nrt_profile(str(output_dir).encode())
            print(f"profile: {n} file(s) written to {output_dir}",
                  file=sys.stderr)

    import antenv
    mod = types.ModuleType("antenv.axon_hooks")
    mod.get_axon_ntff_profile_hook = lambda: _hook
    mod.set_axon_ntff_profile_hook = lambda h: None
    _sys.modules["antenv.axon_hooks"] = mod
    antenv.axon_hooks = mod
    return True


def kernel(**inputs):
    global LAST_RESULTS, LAST_EXEC_NS
    import ml_dtypes
    from concourse.bass_utils import run_bass_kernel_spmd

    inputs = {k: np.asarray(v) for k, v in inputs.items()}
    weights = _prep_weights(inputs)

    ct = np.asarray(inputs["ct_feat"], dtype=np.float32)
    xs = ct.astype(ml_dtypes.bfloat16).reshape(NCORES, BC, C_IN, N)

    in_maps = [dict(weights, x=np.ascontiguousarray(xs[i]))
               for i in range(NCORES)]

    nc = _get_program()
    trace = bool(int(os.environ.get("CK_PROFILE", "0")))
    if trace:
        trace = _ensure_ntff_hook()
    res = run_bass_kernel_spmd(nc, in_maps, list(range(NCORES)), trace=trace)
    LAST_RESULTS = res
    LAST_EXEC_NS = res.exec_time_ns

    out = np.concatenate([np.asarray(res.results[i]["out"])
                          for i in range(NCORES)], axis=0)
    return out.astype(np.float32)


# revision 7
# speedup vs baseline: 2.5188x; 1.0482x over previous
"""Trainium2 Bass kernel: CenterHeadIoU 1x1-conv stack (bf16 v3, PE-tiled).

Computes, for x = ct_feat [B=32, C=128, N=8192]:
  y = relu(bn(sh_w @ x))                       [B, 64, N]
  z_h = relu(bn_h(head_w1[h] @ y)), h=0..5     [B, 64, N] each
  out = concat_h(head_final_w[h] @ z_h + b_h)  [B, 12, N]

Sharding: data-parallel over batch, 4 batches per core on 8 cores;
weights tiny and replicated. BN folded on the host; x and weights cast
to bf16 (rel err ~7e-4 vs the 2e-2 gate). On this part the PE streams
512 columns in ~427ns regardless of dtype (1.2 GHz), so the design
minimizes full-width matmul slots via PE array tiling (concurrent
tiles at distinct tile_positions):

Per group of 4 tiles (F=512 cols each), two mode stretches:
  (128,32): mm1 for the NEXT group — 8 col-tiled M=32 matmuls, 4 per
       slot (2 tiles share a slot), into one [128,2F] psum tile; ONE
       ACT op evacuates the whole group's y (relu+bias). Then mm3 for
       the PREVIOUS group — 3 waves of 4 col-tiled accumulating
       matmuls (M=12 at positions 0/32/64/96) pack 4 tiles' outputs
       into ONE psum bank (z is a group old, so no PE stall); a single
       copy op evacuates it, then 4 out-DMAs on the idle GpSimd queue.
  (64,128): mm2 — pair p of tile t runs on row half t%2 (y(t) lives in
       sbuf partitions 0-63 or 64-127), so two tiles' pair-matmuls run
       concurrently: 12 matmuls in ~6 slots, each tile-pair sharing
       [128,2F] psum tiles (row halves write different banks). K=64
       (no bias row): the first-layer bias is applied by the merged
       [128,2F] PSUM-evac ops (per-partition bias AP, 3 per pair,
       alternating ACT/DVE). The stretch ends with a rank-1 "bias
       matmul" (K=64 onesK row) that fills the NEXT out bank with b3
       and sets has_written everywhere, so mm3 waves accumulate with
       start=False and the final evac is a plain copy.
All 8 PSUM allocations per group share one 4x[128,2F] ring (tag "ps").
A post-pass moves multi-wait sync conditions onto single-wait NoOp
carriers (this walrus build caps sync waits per instruction).
"""

import os
import sys
import numpy as np

B, C_IN, N, HC = 32, 128, 8192, 64
NCORES = 8
BC = B // NCORES            # batches per core
F = 512                     # free-dim tile = one fp32 PSUM bank
NT = N // F                 # tiles per batch
T = BC * NT                 # tiles per core
G = 4                       # tiles per out-group (4 col positions)
NG = T // G
EPS = 1e-5
HEAD_OUT = [3, 2, 1, 3, 2, 1]        # hm, reg, height, dim, rot, iou
PAIR_OFF = [0, 5, 9]                 # channel offset of pair p in the 12-ch output

_CACHE = {}
LAST_RESULTS = None
LAST_EXEC_NS = None


def _build_program():
    import concourse.bass as bass
    import concourse.mybir as mybir
    import concourse.tile as tile

    f32 = mybir.dt.float32
    bf16 = mybir.dt.bfloat16
    AF = mybir.ActivationFunctionType
    OP = mybir.AluOpType

    nc = bass.Bass("TRN2", target_bir_lowering=False, debug=False,
                   num_devices=NCORES)

    x = nc.dram_tensor("x", [BC, C_IN, N], bf16, kind="ExternalInput").ap()
    w1 = nc.dram_tensor("w1", [C_IN, 128], bf16, kind="ExternalInput").ap()
    b1 = nc.dram_tensor("b1", [128, 1], f32, kind="ExternalInput").ap()
    w2 = nc.dram_tensor("w2", [C_IN, 384], bf16, kind="ExternalInput").ap()
    b2 = nc.dram_tensor("b2", [128, 3], f32, kind="ExternalInput").ap()
    w3 = nc.dram_tensor("w3", [C_IN, 36], bf16, kind="ExternalInput").ap()
    b3r = nc.dram_tensor("b3r", [HC, 128], bf16, kind="ExternalInput").ap()
    onesk = nc.dram_tensor("onesk", [HC, F], bf16, kind="ExternalInput").ap()
    out = nc.dram_tensor("out", [BC, 12, N], f32, kind="ExternalOutput").ap()

    with tile.TileContext(nc) as tc:
        with (
            tc.tile_pool(name="consts", bufs=1) as cpool,
            tc.tile_pool(name="xin", bufs=3) as xpool,
            tc.tile_pool(name="ysb", bufs=2) as ypool,
            tc.tile_pool(name="zsb", bufs=5) as zpool,
            tc.tile_pool(name="osb", bufs=2) as opool,
            tc.tile_pool(name="ps", bufs=4, space="PSUM") as ppool,
        ):
            w1_t = cpool.tile([C_IN, 128], bf16, name="w1_t")
            nc.sync.dma_start(out=w1_t[:], in_=w1[:])
            b1_t = cpool.tile([128, 1], f32, name="b1_t")
            nc.sync.dma_start(out=b1_t[:], in_=b1[:])
            w2_t = cpool.tile([C_IN, 384], bf16, name="w2_t")
            nc.sync.dma_start(out=w2_t[:], in_=w2[:])
            b2_t = cpool.tile([128, 3], f32, name="b2_t")
            nc.sync.dma_start(out=b2_t[:], in_=b2[:])
            w3_t = cpool.tile([C_IN, 36], bf16, name="w3_t")
            nc.sync.dma_start(out=w3_t[:], in_=w3[:])
            b3r_t = cpool.tile([HC, 128], bf16, name="b3r_t")
            nc.sync.dma_start(out=b3r_t[:], in_=b3r[:])
            onesk_t = cpool.tile([HC, F], bf16, name="onesk_t")
            nc.sync.dma_start(out=onesk_t[:], in_=onesk[:])

            xts = {}
            yts = {}
            zsps = {}
            pos = {}

            def psum():
                return ppool.tile([C_IN, 2 * F], f32, name="ps", tag="ps")

            def load_group(g):
                if g >= NG:
                    return
                b, jg = divmod(g, NT // G)
                xt = xpool.tile([C_IN, G * F], bf16, name="xt", tag="xt")
                nc.sync.dma_start(out=xt[:],
                                  in_=x[b, :, jg * G * F:(jg + 1) * G * F])
                xts[g] = xt

            def mm1_group(g):
                # (128,32) stretch (shared with mm3): y for group g, two
                # tiles per slot via 4-way col tiling into one [128,2F]
                # psum tile; ONE ACT op then evacuates the whole group's y.
                if g >= NG:
                    return
                xt = xts[g]
                py = psum()
                for pair in range(2):       # tiles (2*pair, 2*pair+1)
                    c0 = pair * F
                    for q in range(4):      # col tiles q0/q32/q64/q96
                        e = q // 2          # tile within the pair
                        nc.tensor.matmul(
                            py[32 * q:32 * (q + 1), c0:c0 + F],
                            w1_t[:, 32 * q:32 * (q + 1)],
                            xt[:, (2 * pair + e) * F:(2 * pair + e + 1) * F],
                            start=True, stop=True,
                            tile_position=(0, 32 * q))
                yt = ypool.tile([C_IN, 2 * F], bf16, name="yt", tag="yt")
                nc.scalar.activation(yt[:, :], py[:, :], AF.Relu,
                                     bias=b1_t[:, 0:1], scale=1.0)
                yts[g] = yt

            def mm2_group(g):
                # (64,128) stretch: pair-matmuls for tiles 4g..4g+3; tile
                # parity picks the row half AND the psum col half, so
                # consecutive tiles overlap and write different banks.
                yt = yts[g]
                for pairidx in range(2):    # tiles (2*pairidx, 2*pairidx+1)
                    c0 = pairidx * F
                    zsp = zpool.tile([C_IN, 6 * F], bf16, name="zsp",
                                     tag="zsp")
                    pzs = []
                    for p in range(3):
                        pz = psum()
                        for e in range(2):
                            h = HC * e
                            nc.tensor.matmul(
                                pz[:, e * F:(e + 1) * F],
                                w2_t[h:h + HC, 128 * p:128 * (p + 1)],
                                yt[h:h + HC, c0:c0 + F],
                                start=True, stop=True)
                        pzs.append(pz)
                    # relu + first-layer bias (per-partition AP, same for
                    # both tiles of the pair); one [128,2F] op per pair p,
                    # alternating ACT/DVE
                    for p in range(3):
                        if (p + pairidx) % 2 == 0:
                            nc.scalar.activation(
                                zsp[:, 2 * p * F:(2 * p + 2) * F],
                                pzs[p][:, :], AF.Relu,
                                bias=b2_t[:, p:p + 1], scale=1.0)
                        else:
                            nc.vector.tensor_scalar(
                                zsp[:, 2 * p * F:(2 * p + 2) * F],
                                pzs[p][:, :], b2_t[:, p:p + 1], 0.0,
                                OP.add, OP.max)
                    zsps[2 * g + pairidx] = zsp
                # rank-1 bias matmul: fills the out bank for group g's mm3
                # with b3 and sets has_written on every element
                po = psum()
                nc.tensor.matmul(po[:, 0:F], b3r_t[:, :], onesk_t[:, :],
                                 start=True, stop=True, skip_group_check=True)
                pos[g] = po

            def mm3_group(g):
                # (128,32) stretch: 3 waves of 4 col-tiled accumulating
                # matmuls; z is one group old so the PE never waits here.
                po = pos.pop(g)
                for p in range(3):
                    for i in range(G):
                        zsp = zsps[2 * g + i // 2]
                        e = i % 2
                        nc.tensor.matmul(
                            po[32 * i:32 * i + 12, 0:F],
                            w3_t[:, 12 * p:12 * (p + 1)],
                            zsp[:, (2 * p + e) * F:(2 * p + e + 1) * F],
                            start=False, stop=(p == 2 and i == G - 1),
                            skip_group_check=True,
                            tile_position=(0, 32 * i))
                return po

            def epi_group(g, po):
                b, jg = divmod(g, NT // G)
                ot = opool.tile([128, F], f32, name="ot", tag="ot")
                nc.vector.tensor_scalar_add(ot[:, :], po[:, 0:F], 0.0)
                for i in range(G):
                    j = jg * G + i
                    nc.gpsimd.dma_start(out=out[b, :, j * F:(j + 1) * F],
                                        in_=ot[32 * i:32 * i + 12, :])
                zsps.pop(2 * g, None)
                zsps.pop(2 * g + 1, None)

            load_group(0)
            load_group(1)
            mm1_group(0)
            for g in range(NG):
                load_group(g + 2)
                mm1_group(g + 1)
                if g >= 1:
                    po = mm3_group(g - 1)
                    epi_group(g - 1, po)
                mm2_group(g)
            po = mm3_group(NG - 1)
            epi_group(NG - 1, po)
    _split_waits(nc)
    return nc


def _split_waits(nc, cap=1):
    """This container's walrus build rejects instructions carrying more than
    a small number of sync waits (matmuls: just one). Move excess waits onto
    single-wait NoOp carriers inserted before the instruction on the same
    engine — semantically identical (conjunction of waits, in-order
    sequencers)."""
    import concourse.mybir as mybir

    k = 0
    for func in nc.m.functions:
        for bb in func.blocks:
            insts = bb.instructions
            out_insts = []
            changed = False
            for inst in insts:
                si = inst.sync_info
                waits = list(si.on_wait) if si and si.on_wait else []
                if len(waits) > cap:
                    for w in waits[:-cap]:
                        d = mybir.InstNoOp(name=f"I-sw{k}", ins=[], outs=[])
                        k += 1
                        d.engine = inst.engine
                        d.sync_info = mybir.SyncInfo(on_wait=[w], on_update=[])
                        nc.register_instruction(d)
                        out_insts.append(d)
                    inst.sync_info = mybir.SyncInfo(
                        on_wait=waits[-cap:],
                        on_update=list(si.on_update) if si.on_update else [])
                    changed = True
                out_insts.append(inst)
            if changed:
                bb.instructions = out_insts


def _get_program():
    if "nc" not in _CACHE:
        _CACHE["nc"] = _build_program()
    return _CACHE["nc"]


def _prep_weights(d):
    """Fold BN into conv weights/biases; pack stationary matrices (bf16)."""
    import ml_dtypes
    f8 = np.float64
    bf = ml_dtypes.bfloat16

    def g(name):
        return np.asarray(d[name], dtype=f8)

    # shared conv + BN
    s1 = g("sh_g") / np.sqrt(g("sh_var") + EPS)                     # [64]
    W1e = g("sh_w") * s1[:, None]                                   # [64,128]
    b1e = g("sh_b") * s1 + g("sh_beta") - g("sh_mean") * s1         # [64]
    w1 = np.zeros((C_IN, 128), f8)
    w1[:, 0:HC] = W1e.T                 # col-pair slot 0 -> psum rows 0..63
    w1[:, HC:128] = W1e.T               # col-pair slot 1 -> psum rows 64..127
    b1 = np.concatenate([b1e, b1e])[:, None]                        # [128,1]

    # head first layers + BN: K=64 row-tiled stationaries; pair p's block
    # has head 2p in output cols 0..63 and head 2p+1 in cols 64..127; the
    # weights are duplicated into both sbuf row halves (T0 and T8 copies).
    s2 = g("head_g1") / np.sqrt(g("head_var1") + EPS)               # [6,64]
    W2e = g("head_w1") * s2[:, :, None]                             # [6,64,64]
    b2e = g("head_b1") * s2 + g("head_beta1") - g("head_mean1") * s2  # [6,64]
    w2 = np.zeros((C_IN, 384), f8)
    b2 = np.zeros((128, 3), f8)
    for p in range(3):
        blk = np.zeros((HC, 128), f8)
        blk[:, 0:HC] = W2e[2 * p].T
        blk[:, HC:128] = W2e[2 * p + 1].T
        w2[0:HC, 128 * p:128 * (p + 1)] = blk
        w2[HC:128, 128 * p:128 * (p + 1)] = blk
        b2[0:HC, p] = b2e[2 * p]
        b2[HC:128, p] = b2e[2 * p + 1]

    # final convs: three accumulating M=12 blocks (pair p: head 2p from z
    # rows 0..63, head 2p+1 from rows 64..127, into its channel offsets)
    names = ["hm", "reg", "height", "dim", "rot", "iou"]
    Wf = [g(n + "_w") for n in names]
    bfin = [g(n + "_b") for n in names]
    w3 = np.zeros((C_IN, 36), f8)
    b3full = np.zeros((12,), f8)
    for p in range(3):
        ha, hb = 2 * p, 2 * p + 1
        ca, cb = HEAD_OUT[ha], HEAD_OUT[hb]
        off = PAIR_OFF[p]
        w3[0:64, 12 * p + off:12 * p + off + ca] = Wf[ha].T
        w3[64:128, 12 * p + off + ca:12 * p + off + ca + cb] = Wf[hb].T
        b3full[off:off + ca] = bfin[ha]
        b3full[off + ca:off + ca + cb] = bfin[hb]
    # rank-1 bias matmul operands: row 0 of b3r x row 0 of onesk broadcasts
    # b3 (in the 4x32 col-group layout) across the whole out bank
    b3r = np.zeros((HC, 128), f8)
    for i in range(G):
        b3r[0, 32 * i:32 * i + 12] = b3full
    onesk = np.zeros((HC, F), f8)
    onesk[0, :] = 1.0

    c = np.float32
    return {"w1": w1.astype(bf), "b1": b1.astype(c), "w2": w2.astype(bf),
            "b2": b2.astype(c), "w3": w3.astype(bf), "b3r": b3r.astype(bf),
            "onesk": onesk.astype(bf)}


def _ensure_ntff_hook():
    """Install the antenv.axon_hooks NTFF-profile shim if the container's
    antenv package lacks it (profiling only; never used in grading runs)."""
    try:
        from antenv.axon_hooks import get_axon_ntff_profile_hook  # noqa: F401
        return True
    except ImportError:
        pass
    import contextlib
    import ctypes
    import sys as _sys
    import types

    so_path = "/opt/axon/libaxon_pjrt.so"
    if not os.path.exists(so_path):
        return False
    lib = ctypes.CDLL(so_path)
    if not hasattr(lib, "axon_start_nrt_profile"):
        return False
    lib.axon_start_nrt_profile.argtypes = [ctypes.POINTER(ctypes.c_int64),
                                           ctypes.c_size_t]
    lib.axon_start_nrt_profile.restype = ctypes.c_int64
    lib.axon_stop_nrt_profile.argtypes = [ctypes.c_char_p]
    lib.axon_stop_nrt_profile.restype = ctypes.c_int64

    @contextlib.contextmanager
    def _hook(output_dir, device_ids):
        import jax
        jax.devices()
        if device_ids:
            ids = (ctypes.c_int64 * len(device_ids))(*device_ids)
            rc = lib.axon_start_nrt_profile(ids, len(device_ids))
        else:
            rc = lib.axon_start_nrt_profile(None, 0)
        if rc != 0:
            raise RuntimeError(f"axon_start_nrt_profile rc={rc}")
        try:
            yield
        finally:
            n = lib.axon_stop_nrt_profile(str(output_dir).encode())
            print(f"profile: {n} file(s) written to {output_dir}",
                  file=sys.stderr)

    import antenv
    mod = types.ModuleType("antenv.axon_hooks")
    mod.get_axon_ntff_profile_hook = lambda: _hook
    mod.set_axon_ntff_profile_hook = lambda h: None
    _sys.modules["antenv.axon_hooks"] = mod
    antenv.axon_hooks = mod
    return True


def kernel(**inputs):
    global LAST_RESULTS, LAST_EXEC_NS
    import ml_dtypes
    from concourse.bass_utils import run_bass_kernel_spmd

    inputs = {k: np.asarray(v) for k, v in inputs.items()}
    weights = _prep_weights(inputs)

    ct = np.asarray(inputs["ct_feat"], dtype=np.float32)
    xs = ct.astype(ml_dtypes.bfloat16).reshape(NCORES, BC, C_IN, N)

    in_maps = [dict(weights, x=np.ascontiguousarray(xs[i]))
               for i in range(NCORES)]

    nc = _get_program()
    trace = bool(int(os.environ.get("CK_PROFILE", "0")))
    if trace:
        trace = _ensure_ntff_hook()
    res = run_bass_kernel_spmd(nc, in_maps, list(range(NCORES)), trace=trace)
    LAST_RESULTS = res
    LAST_EXEC_NS = res.exec_time_ns

    out = np.concatenate([np.asarray(res.results[i]["out"])
                          for i in range(NCORES)], axis=0)
    return out.astype(np.float32)
